# revision 51
# baseline (speedup 1.0000x reference)
"""Trainium2 Bass kernel for a 2-layer dense-graph GAT encoder (N=4096, H=4).

Math: attention scores are additive: e[i,j,h] = lrelu(e_src[i,h] + e_tgt[j,h]).
exp(lrelu(s)) with s = es + et factors as
    exp(0.2*es) * [ c * max(1, u*v) ],   u = exp(0.8*es_i), v = exp(0.8*et_j),
    c = exp(0.2*et_j),
and the exp(0.2*es_i) factor cancels in the softmax.  So each (j,i) attention
tile is ONE DVE tensor_scalar op:  T[j,i] = max(c_j, (c_j*v_j)*u_i)  applied to
a broadcast tile of u — no N^2 transcendentals.  The N^2 work left is one DVE
op + one PE matmul per 128x512 tile.

Sharding: rows (queries) are split 512/core across 8 cores.  Layer-1
projections (x @ W1) are computed replicated from a pre-transposed x; the
layer-1 output shard h^T (256x512) is AllGathered between layers; layer-2
projections are recomputed replicated from the gathered h^T.  Final output is
returned per-core as (512, 256) row shards and concatenated on host.
"""

import numpy as np
import ml_dtypes

N = 4096
NCORES = 8
NS = N // NCORES          # 512 rows per core
H = 4
D1 = 64                   # layer-1 head dim
HID = 256                 # hidden = H*D1, layer-2 head dim
K1 = 128                  # state_dim
NT = N // 128             # 32 j-tiles
LN_EPS = 1e-5
SW2E = 16.0               # fp8 scale for the layer-2 score columns
WXS = 256.0               # fp8 scale for wx2 (layer-2 V-side)
S2 = 64.0                 # fp8 scale for T2 tiles (cancels in num/den)
# layer-2 j-tiles generated on ScalarE in relu/deviation form (13 of 32)
ACT2 = (1, 3, 6, 8, 11, 13, 16, 18, 21, 23, 26, 28, 31)

_BF = ml_dtypes.bfloat16
_F8 = ml_dtypes.float8_e4m3

_compiled = None
_DEBUG = False


def _build():
    import concourse.bass as bass
    import concourse.mybir as mybir
    import concourse.tile as tile
    from concourse import bacc

    # All ACT functions used here (Exp, Ln, Copy, Relu, Square, Identity)
    # live in the natural_log_exp_and_others set; prefer it so the table is
    # loaded once instead of thrashing Ln<->Exp sets in the LN tail.
    if not getattr(bacc, "_ant_act_tables_patched", False):
        _orig_gat = bacc.get_activation_tables

        def _pref_tables(arch):
            tabs = dict(_orig_gat(arch))
            pref = "natural_log_exp_and_others"
            if pref in tabs:
                # keep entry ORDER (act_func_set_id is positional) but hide
                # every other set's functions so the picker lands on pref
                tabs = {k: (v if k == pref else set())
                        for k, v in tabs.items()}
            return tabs

        bacc.get_activation_tables = _pref_tables
        bacc._ant_act_tables_patched = True

    f32 = mybir.dt.float32
    bf16 = mybir.dt.bfloat16
    f8 = mybir.dt.float8e4
    AF = mybir.ActivationFunctionType
    OP = mybir.AluOpType
    PM = mybir.MatmulPerfMode

    nc = bacc.Bacc("TRN2", target_bir_lowering=False, debug=False,
                   num_devices=NCORES)

    # ---- I/O ----
    xT_d = nc.dram_tensor("xT", [K1, N], bf16, kind="ExternalInput")
    xTm_d = nc.dram_tensor("xTm", [K1, NS], bf16, kind="ExternalInput")
    w1_d = nc.dram_tensor("w1a", [K1, HID + 2 * H], bf16, kind="ExternalInput")
    w2_d = nc.dram_tensor("w2a", [HID, H * HID], bf16, kind="ExternalInput")
    w2e8_d = nc.dram_tensor("w2e8", [K1, 2, 2 * H], f8, kind="ExternalInput")
    gb_d = nc.dram_tensor("gb", [2, HID], f32, kind="ExternalInput")
    out_d = nc.dram_tensor("outT", [NS, HID], f32, kind="ExternalOutput")
    if _DEBUG:
        dbg_esb1_d = nc.dram_tensor("dbg_esb1", [128, NT, 8], f32,
                                    kind="ExternalOutput")
        dbg_esb2_d = nc.dram_tensor("dbg_esb2", [128, NT, 8], f32,
                                    kind="ExternalOutput")
        dbg_h1_d = nc.dram_tensor("dbg_h1", [128, 2, NS], f32,
                                  kind="ExternalOutput")
        dbg_o2_d = nc.dram_tensor("dbg_o2", [NS, HID], f32,
                                  kind="ExternalOutput")

    W1C = HID + 2 * H        # 264
    W2C = H * HID + 2 * H    # 1032

    with tile.TileContext(nc) as tc:
        with (
            tc.tile_pool(name="persist", bufs=1) as pp,
            tc.tile_pool(name="xpool", bufs=1) as xp,
            tc.tile_pool(name="work", bufs=1) as wp,
            tc.tile_pool(name="tp", bufs=8) as tp,
            tc.tile_pool(name="dram", bufs=1, space="DRAM") as dram,
        ):
            # tiny dummy collective FIRST so the CC firmware warm-up
            # (barrier + HAM setup, ~55us) starts as early as possible and
            # runs under layer-1 compute.
            warm_in = dram.tile([1, 64], bf16)
            warm_out = dram.tile([NCORES, 64], bf16, addr_space="Shared")
            warm_sb = wp.tile([1, 64], bf16, tag="warm")
            nc.vector.memset(warm_sb[:], 0.0)
            nc.sync.dma_start(warm_in[:], warm_sb[:])
            nc.gpsimd.collective_compute(
                "AllGather", OP.bypass,
                replica_groups=[list(range(NCORES))],
                ins=[warm_in.opt()], outs=[warm_out.opt()])

            # ---------- loads ----------
            xT = xp.tile([K1, N], bf16)
            xTm = xp.tile([K1, NS], bf16)
            w1 = pp.tile([K1, W1C], bf16)
            w2 = pp.tile([K1, 2, H * HID], bf16)   # k-tiles of W2 rows
            nc.sync.dma_start(w1[:], w1_d[:])
            nc.sync.dma_start(xTm[:], xTm_d[:])
            for q in range(4):  # chunked for DMA parallelism
                nc.sync.dma_start(xT[:, q * 1024:(q + 1) * 1024],
                                  xT_d[:, q * 1024:(q + 1) * 1024])
            # w2/w2e8 are not needed until the layer-2 projection — load
            # them after the layer-1 operands so phase A starts sooner
            nc.sync.dma_start(w2[:, 0, :], w2_d[0:128, :])
            nc.sync.dma_start(w2[:, 1, :], w2_d[128:256, :])
            w2e8 = pp.tile([K1, 2, 2 * H], f8)
            nc.sync.dma_start(w2e8[:], w2e8_d[:])

            # gamma/beta broadcast rows
            g_row = pp.tile([1, HID], f32)
            b_row = pp.tile([1, HID], f32)
            nc.sync.dma_start(g_row[:], gb_d[0:1, :])
            nc.sync.dma_start(b_row[:], gb_d[1:2, :])
            g_brc = pp.tile([128, HID], f32)
            b_brc = pp.tile([128, HID], f32)
            nc.gpsimd.partition_broadcast(g_brc[:], g_row[:])
            nc.gpsimd.partition_broadcast(b_brc[:], b_row[:])

            # ---------- persistent layer-1 state ----------
            wx1 = pp.tile([128, NT, H, D1 + 1], bf16)     # [.., 0:64]=Wx, 64=ones
            nc.vector.memset(wx1[:, :, :, D1], 1.0)
            esb1 = wp.tile([128, NT, 8], f32, tag="esb1")
            c1 = pp.tile([128, NT, H], f32)
            cv1 = pp.tile([128, NT, H], f32)

            with tc.tile_pool(name="psA", bufs=4, space="PSUM") as psA:
                for jt in range(NT):
                    pA = psA.tile([128, W1C], f32, tag="pA")
                    nc.tensor.matmul(pA[:], xT[:, jt * 128:(jt + 1) * 128],
                                     w1[:], start=True, stop=True)
                    if jt % 2 == 0:
                        nc.vector.tensor_copy(wx1[:, jt, :, 0:D1],
                                              pA[:, 0:HID])
                    else:
                        nc.scalar.copy(wx1[:, jt, :, 0:D1], pA[:, 0:HID])
                    nc.vector.tensor_copy(esb1[:, jt, :], pA[:, HID:W1C])

                # u1 rows for my shard: e_src1^T via M=1 matmuls
                u1row = []
                for h in range(H):
                    pu = psA.tile([1, NS], f32, tag="pu", bufs=2)
                    nc.tensor.matmul(pu[:], w1[:, HID + h:HID + h + 1],
                                     xTm[:], start=True, stop=True)
                    ur = pp.tile([1, NS], bf16, name=f"u1row{h}",
                                 tag=f"u1row{h}")
                    nc.scalar.activation(ur[:], pu[:], AF.Exp, scale=0.8)
                    u1row.append(ur)

            uv1 = wp.tile([128, NT, 8], f32, tag="uv1")
            nc.scalar.activation(uv1[:], esb1[:], AF.Exp, scale=0.8)
            nc.scalar.activation(c1[:], esb1[:, :, H:2 * H], AF.Exp, scale=0.2)
            nc.vector.tensor_tensor(cv1[:], uv1[:, :, H:2 * H], c1[:], OP.mult)
            # for the ACT-generated (relu-form) head: -c bias + bf16 c column
            nc1 = pp.tile([128, NT, H], f32)
            nc.vector.tensor_scalar(nc1[:], c1[:], -1.0, None, OP.mult)
            c1b = pp.tile([128, NT, H], bf16)
            nc.vector.tensor_copy(c1b[:], c1[:])
            ones_row = pp.tile([1, NS], bf16)
            nc.vector.memset(ones_row[:], 1.0)

            bu1 = pp.tile([128, H, NS], bf16)
            for h in range(H):
                nc.gpsimd.partition_broadcast(bu1[:, h, :], u1row[h][:])

            # ---------- phase B: layer-1 attention for my 512 rows ----------
            # tiles with jt % 4 == 3 are generated on ScalarE in relu-form:
            #   R = relu(cv*u - c) = c*(max(1, u*v) - 1)
            # their matmuls miss sum_{those j} c_j*Wx_aug[j,:], injected via
            # a K=1 matmul of the per-head correction row C1_h.
            ACT_JT = [jt for jt in range(NT) if jt % 3 == 1]
            # single bounce for all 4 heads: [p, kt, i] (kt: head pair)
            bounce = dram.tile([128, 2, NS], f8)
            gat = dram.tile([NCORES, 128, 2, NS], f8, addr_space="Shared")
            hallT = pp.tile([128, 2, NCORES, NS], f8)
            eluhs = []
            with tc.tile_pool(name="psB", bufs=1, space="PSUM") as psB:
                # correction rows: C1_h = sum_{jt in ACT_JT} c_j*Wx_aug[j,:]
                c1s = []
                for h in range(H):
                    pC1 = psB.tile([1, D1 + 1], f32, tag="pC1", bufs=2)
                    for i, jt in enumerate(ACT_JT):
                        nc.tensor.matmul(pC1[:], c1b[:, jt, h:h + 1],
                                         wx1[:, jt, h, :],
                                         start=(i == 0),
                                         stop=(i == len(ACT_JT) - 1))
                    cs = wp.tile([1, D1 + 1], bf16, name=f"c1s{h}",
                                 tag=f"c1s{h}")
                    nc.vector.tensor_copy(cs[:], pC1[:])
                    c1s.append(cs)

                def _epi1(h, pB):
                    # epilogue: h1 = elu(num/den), DMA into bounce half
                    # o row 64 doubles as the den staging row
                    o = wp.tile([D1 + 1, NS], f32, tag="o", bufs=2)
                    nc.vector.tensor_copy(o[D1:D1 + 1, :], pB[D1:D1 + 1, :])
                    den = wp.tile([1, NS], f32, tag="den", bufs=2)
                    nc.sync.dma_start(den[:], o[D1:D1 + 1, :])
                    denr = wp.tile([1, NS], f32, tag="denr", bufs=2)
                    nc.vector.reciprocal_approx_fast(denr[:], den[:])
                    brc = wp.tile([D1, NS], f32, tag="brc", bufs=2)
                    nc.gpsimd.partition_broadcast(brc[:], denr[:])
                    nc.vector.tensor_tensor(o[0:D1, :], pB[0:D1, :], brc[:],
                                            OP.mult)
                    # elu(x) = (relu(x) - 1) + exp(min(x, 0))
                    tmn = wp.tile([D1, NS], f32, tag="tmn", bufs=2)
                    nc.vector.tensor_scalar(tmn[:], o[0:D1, :], 0.0, None,
                                            OP.min)
                    nc.scalar.activation(tmn[:], tmn[:], AF.Exp)  # in place
                    trl = wp.tile([D1, NS], f32, tag="trl", bufs=2)
                    nc.vector.tensor_scalar(trl[:], o[0:D1, :], 0.0, -1.0,
                                            OP.max, OP.add)
                    eluh = wp.tile([D1, NS], f8, name=f"eluh{h}",
                                   tag=f"eluh{h}")
                    nc.vector.tensor_tensor(eluh[:], tmn[:], trl[:], OP.add)
                    eluhs.append(eluh)
                    nc.sync.dma_start(
                        bounce[(h % 2) * D1:(h % 2 + 1) * D1, h // 2, :],
                        eluh[:])
                    # one gather for all heads: avoids the ~8us CC
                    # firmware gap between back-to-back collectives
                    if h == 3:
                        nc.gpsimd.collective_compute(
                            "AllGather", OP.bypass,
                            replica_groups=[list(range(NCORES))],
                            ins=[bounce.opt()], outs=[gat.opt()])
                        # split the rearrange per source core so the
                        # projection's first j-tiles (core 0) start ~3us
                        # before the whole 1MB reshuffle lands
                        for cc in range(NCORES):
                            nc.sync.dma_start(hallT[:, :, cc, :], gat[cc])

                for h in range(H):
                    pB = psB.tile([D1 + 1, NS], f32, name=f"pB{h}",
                                  tag=f"pB{h}")
                    # inject correction row (opens the accumulation group)
                    nc.tensor.matmul(pB[:], c1s[h][:], ones_row[:],
                                     start=True, stop=False)
                    for jt in range(NT):
                        t1 = tp.tile([128, NS], bf16, tag="T1")
                        if jt % 3 == 1:
                            nc.scalar.activation(
                                t1[:], bu1[:, h, :], AF.Relu,
                                bias=nc1[:, jt, h:h + 1],
                                scale=cv1[:, jt, h:h + 1])
                        else:
                            nc.vector.tensor_scalar(
                                t1[:], bu1[:, h, :], cv1[:, jt, h:h + 1],
                                c1[:, jt, h:h + 1], OP.mult, OP.max)
                        nc.tensor.matmul(
                            pB[:], wx1[:, jt, h, :], t1[:],
                            start=False,
                            stop=(jt == NT - 1))
                    _epi1(h, pB)

            # my own h^T back from local bounce (for u2 rows)
            hmT = wp.tile([128, 2, NS], f8)
            nc.sync.dma_start(hmT[:], bounce[:])

            if _DEBUG:
                dbg_h1 = wp.tile([128, 2, NS], f8)
                nc.sync.dma_start(dbg_h1[:], bounce[:])
                dbg_h1f = wp.tile([128, 2, NS], f32)
                nc.vector.tensor_copy(dbg_h1f[:], dbg_h1[:])
                nc.sync.dma_start(dbg_h1_d[:], dbg_h1f[:])

            # ---------- persistent layer-2 state ----------
            # wx2 holds Wx2_aug * WXS in fp8, laid out for DoubleRow matmuls:
            # [p, jt2, k, h, col] = Wx2_aug[jt2*256 + k*128 + p, h, col]
            NT2 = NT // 2
            wx2 = pp.tile([128, NT2, 2, H, HID + 1], f8)
            nc.vector.memset(wx2[:, :, :, :, HID], 1.0)  # den col: NOT scaled
            esb2 = wp.tile([128, NT, 8], f32, tag="esb1")
            c2 = pp.tile([128, NT, H], f32)
            cv2 = pp.tile([128, NT, H], f32)

            # keep-warm + u2 rows: the PE idles ~37us between the end of
            # phase B and the projection start, which drops the HAM clock
            # gate to K=4/8 and makes the first chunk of the projection run
            # at half clock.  Burn junk matmuls (gated on head 0's epilogue
            # output so they slot in right as phase B's matmuls drain) to
            # hold K=8/8 through the gather, with the u2row matmuls
            # sandwiched so they still run as soon as hmT lands.
            u2row = []
            with tc.tile_pool(name="psU", bufs=1, space="PSUM") as psU:
                pw = psU.tile([128, NS], f32, tag="pw", bufs=1)
                for i in range(45):
                    nc.tensor.matmul(pw[:], eluhs[0][:, 0:128],
                                     eluhs[0][:, :], start=True, stop=True)
                for h in range(H):
                    pu2 = psU.tile([1, NS], f32, tag="pu2", bufs=1)
                    for kt in range(2):
                        nc.tensor.matmul(pu2[:], w2e8[:, kt, h:h + 1],
                                         hmT[:, kt, :], start=(kt == 0),
                                         stop=(kt == 1))
                    ur2 = pp.tile([1, NS], bf16, name=f"u2row{h}",
                                  tag=f"u2row{h}")
                    nc.scalar.activation(ur2[:], pu2[:], AF.Exp,
                                         scale=0.8 / SW2E)
                    u2row.append(ur2)
                for i in range(115):
                    nc.tensor.matmul(pw[:], eluhs[0][:, 0:128],
                                     eluhs[0][:, :], start=True, stop=True)

            bu2 = pp.tile([128, H, NS], bf16)
            for h in range(H):
                nc.gpsimd.partition_broadcast(bu2[:, h, :], u2row[h][:])

            # per-jt attention scalars, filled chunk-wise during the proj
            # loop so phase-E T-gen can start while proj is still running
            uv2 = wp.tile([128, NT, 8], f32, tag="uv1")
            cvS2 = pp.tile([128, NT, H], f32)
            cS2 = pp.tile([128, NT, H], f32)
            ncS2 = pp.tile([128, NT, H], f32)
            cS2b = pp.tile([128, NT, H], bf16)
            cs4 = wp.tile([1, H, HID + 1], bf16)  # ACT-set correction rows

            with tc.tile_pool(name="psD", bufs=1, space="PSUM") as psD:
                # Wx2_aug replicated: all 4096 rows
                for jt in range(NT):
                    c8, io = divmod(jt, NT // NCORES)
                    jt2, kk = divmod(jt, 2)
                    pD1 = psD.tile([128, 2, HID], f32, tag="pD1", bufs=3)
                    pD2 = psD.tile([128, 2, HID], f32, tag="pD2", bufs=3)
                    pD3 = psD.tile([128, 8], f32, tag="pD3", bufs=2)
                    lhs2 = hallT[:, :, c8, io * 128:(io + 1) * 128]
                    for kt in range(2):
                        lhs = hallT[:, kt, c8, io * 128:(io + 1) * 128]
                        st, sp = (kt == 0), (kt == 1)
                        nc.tensor.matmul(pD1[:], lhs, w2[:, kt, 0:512],
                                         start=st, stop=sp)
                        nc.tensor.matmul(pD2[:], lhs, w2[:, kt, 512:1024],
                                         start=st, stop=sp)
                    nc.tensor.matmul(pD3[:], lhs2, w2e8[:],
                                     start=True, stop=True,
                                     perf_mode=PM.DoubleRow)
                    # esb2 copy first so pD3 recycles promptly
                    nc.vector.tensor_copy(esb2[:, jt, :], pD3[:])
                    if jt % 2 == 0:
                        nc.vector.tensor_scalar(wx2[:, jt2, kk, 0:2, 0:HID],
                                                pD1[:], WXS, None, OP.mult)
                        nc.scalar.mul(wx2[:, jt2, kk, 2:4, 0:HID], pD2[:],
                                      WXS)
                    else:
                        nc.scalar.mul(wx2[:, jt2, kk, 0:2, 0:HID], pD1[:],
                                      WXS)
                        nc.vector.tensor_scalar(wx2[:, jt2, kk, 2:4, 0:HID],
                                                pD2[:], WXS, None, OP.mult)
                    if jt % 8 == 7:
                        # chunk of 8 jt finished: derive attention scalars
                        # (esb2 carries an extra SW2E factor from w2e8)
                        sl = slice(jt - 7, jt + 1)
                        nc.scalar.activation(uv2[:, sl, :], esb2[:, sl, :],
                                             AF.Exp, scale=0.8 / SW2E)
                        nc.scalar.activation(c2[:, sl, :],
                                             esb2[:, sl, H:2 * H],
                                             AF.Exp, scale=0.2 / SW2E)
                        nc.vector.tensor_tensor(cv2[:, sl, :],
                                                uv2[:, sl, H:2 * H],
                                                c2[:, sl, :], OP.mult)
                        nc.vector.tensor_scalar(cvS2[:, sl, :], cv2[:, sl, :],
                                                S2, None, OP.mult)
                        nc.vector.tensor_scalar(cS2[:, sl, :], c2[:, sl, :],
                                                S2, None, OP.mult)
                        nc.vector.tensor_scalar(ncS2[:, sl, :], c2[:, sl, :],
                                                -S2, None, OP.mult)
                        nc.vector.tensor_copy(cS2b[:, sl, :], cS2[:, sl, :])

            # correction rows: C2_h = sum_{jt in ACT2} S2*c_j * wx2[j,h,:]
            # (consumed by the inject-last matmuls, so running these after
            # the proj loop costs nothing on the critical path)
            with tc.tile_pool(name="psC", bufs=1, space="PSUM") as psC:
                for h in range(H):
                    pC2 = psC.tile([1, HID + 1], f32, tag="pC2", bufs=2)
                    for i, ja in enumerate(ACT2):
                        nc.tensor.matmul(pC2[:], cS2b[:, ja, h:h + 1],
                                         wx2[:, ja // 2, ja % 2, h, :],
                                         start=(i == 0),
                                         stop=(i == len(ACT2) - 1))
                    nc.vector.tensor_copy(cs4[:, h, :], pC2[:])

            # ---------- phase E: layer-2 attention + LN ----------
            # head-outer, full i width; per head 4 psum tiles (one per
            # 128-row i-subtile); two heads' tile sets rotate in PSUM.
            # fp8 T2 (scaled by S2) against fp8 wx2 (scaled by WXS) with
            # DoubleRow j-pair matmuls; jt in ACT2 generated on ScalarE in
            # relu (deviation) form, rest on DVE in direct max form.
            o2acc = [wp.tile([128, HID], f32, name=f"o2acc{m}",
                             tag=f"o2acc{m}") for m in range(4)]
            eps_c = pp.tile([128, 1], f32)
            nc.vector.memset(eps_c[:], LN_EPS)

            def _tail(m):
                # elu + layernorm over features for one 128-row m-chunk.
                # exp(min(x,0)) = exp(-relu(-x)): two ACT ops, no DVE min;
                # mean/var via bn_stats/bn_aggr in two DVE ops.
                o2 = o2acc[m]
                eex = wp.tile([128, HID], f32, tag="eex", bufs=2)
                nc.scalar.activation(eex[:], o2[:], AF.Relu, scale=-1.0)
                nc.scalar.activation(eex[:], eex[:], AF.Exp, scale=-1.0)
                erl = wp.tile([128, HID], f32, tag="erl", bufs=2)
                nc.vector.tensor_scalar(erl[:], o2[:], 0.0, -1.0,
                                        OP.max, OP.add)
                he = wp.tile([128, HID], f32, tag="he", bufs=2)
                nc.vector.tensor_tensor(he[:], eex[:], erl[:], OP.add)
                st6 = wp.tile([128, 6], f32, tag="st6", bufs=2)
                nc.vector.bn_stats(st6[:], he[:])
                mv = wp.tile([128, 2], f32, tag="mv", bufs=2)
                nc.vector.bn_aggr(mv[:], st6[:])
                lnv = wp.tile([128, 1], f32, tag="lnv", bufs=2)
                nc.scalar.activation(lnv[:], mv[:, 1:2], AF.Ln,
                                     bias=eps_c[:])
                rstd = wp.tile([128, 1], f32, tag="rstd", bufs=2)
                nc.scalar.activation(rstd[:], lnv[:], AF.Exp, scale=-0.5)
                xn = wp.tile([128, HID], f32, tag="xn", bufs=2)
                nc.vector.tensor_scalar(xn[:], he[:], mv[:, 0:1], rstd[:],
                                        OP.subtract, OP.mult)
                y = wp.tile([128, HID], f32, tag="y", bufs=2)
                nc.vector.tensor_tensor(y[:], xn[:], g_brc[:], OP.mult)
                outt = wp.tile([128, HID], f32, tag="outt", bufs=2)
                nc.vector.tensor_tensor(outt[:], y[:], b_brc[:], OP.add)
                r0 = m * 128
                nc.sync.dma_start(out_d[r0:r0 + 128, :], outt[:])
                if _DEBUG:
                    nc.sync.dma_start(dbg_o2_d[r0:r0 + 128, :], o2[:])

            with tc.tile_pool(name="psE", bufs=1, space="PSUM") as psE:
                def _gen_row(h):
                    # T2 tiles for one head (issued one head ahead so the
                    # last head's LN chains never queue behind T-gen)
                    row = []
                    for jt2 in range(NT2):
                        t2 = tp.tile([128, 2, NS], f8, tag="T2", bufs=32)
                        row.append(t2)
                        for k in range(2):
                            jt = jt2 * 2 + k
                            if jt in ACT2:
                                nc.scalar.activation(
                                    t2[:, k, :], bu2[:, h, :], AF.Relu,
                                    bias=ncS2[:, jt, h:h + 1],
                                    scale=cvS2[:, jt, h:h + 1])
                            else:
                                nc.vector.tensor_scalar(
                                    t2[:, k, :], bu2[:, h, :],
                                    cvS2[:, jt, h:h + 1],
                                    cS2[:, jt, h:h + 1], OP.mult, OP.max)
                    return row

                nextrow = _gen_row(0)
                for h in range(H):
                    pE = [psE.tile([128, HID + 1], f32,
                                   name=f"pE{h}_{m}", tag=f"pE{h % 2}_{m}")
                          for m in range(4)]
                    t2s = nextrow
                    if h < H - 1:
                        for jt2 in range(NT2):
                            for m in range(4):
                                nc.tensor.matmul(
                                    pE[m][:],
                                    t2s[jt2][:, :, m * 128:(m + 1) * 128],
                                    wx2[:, jt2, :, h, :],
                                    start=(jt2 == 0), stop=False,
                                    perf_mode=PM.DoubleRow)
                        nextrow = _gen_row(h + 1)
                    if h == H - 1:
                        # last head goes m-major so each m-chunk's inject,
                        # epilogue and LN tail overlap the next chunk's
                        # matmuls instead of all landing after the last one
                        for m in range(4):
                            for jt2 in range(NT2):
                                nc.tensor.matmul(
                                    pE[m][:],
                                    t2s[jt2][:, :, m * 128:(m + 1) * 128],
                                    wx2[:, jt2, :, h, :],
                                    start=(jt2 == 0), stop=False,
                                    perf_mode=PM.DoubleRow)
                            nc.tensor.matmul(pE[m][:], ones_row[:, 0:128],
                                             cs4[:, h, :], start=False,
                                             stop=True)
                            dpk1 = wp.tile([128, 1], f32, tag="dpk1",
                                           bufs=2)
                            nc.vector.tensor_scalar(
                                dpk1[:], pE[m][:, HID:HID + 1],
                                float(H) * WXS, None, OP.mult)
                            rr1 = wp.tile([128, 1], f32, tag="rr1", bufs=2)
                            nc.vector.reciprocal_approx_fast(rr1[:],
                                                             dpk1[:])
                            nc.vector.scalar_tensor_tensor(
                                o2acc[m][:], pE[m][:, 0:HID],
                                rr1[:], o2acc[m][:], OP.mult, OP.add)
                            _tail(m)
                        continue
                    # inject the ACT-set correction row last (closes the
                    # group) — lets the jt2 matmuls start before the
                    # correction rows are ready
                    for m in range(4):
                        nc.tensor.matmul(pE[m][:], ones_row[:, 0:128],
                                         cs4[:, h, :], start=False,
                                         stop=True)
                    # divide by H*WXS*den and accumulate the head mean
                    dpk = wp.tile([128, 4], f32, tag="dpk", bufs=2)
                    for m in range(4):
                        nc.vector.tensor_scalar(
                            dpk[:, m:m + 1], pE[m][:, HID:HID + 1],
                            float(H) * WXS, None, OP.mult)
                    rr = wp.tile([128, 4], f32, tag="rr", bufs=2)
                    nc.vector.reciprocal_approx_fast(rr[:], dpk[:])
                    for m in range(4):
                        if h == 0:
                            nc.vector.tensor_scalar(
                                o2acc[m][:], pE[m][:, 0:HID],
                                rr[:, m:m + 1], None, OP.mult)
                        else:
                            nc.vector.scalar_tensor_tensor(
                                o2acc[m][:], pE[m][:, 0:HID],
                                rr[:, m:m + 1], o2acc[m][:],
                                OP.mult, OP.add)
            if _DEBUG:
                nc.sync.dma_start(dbg_esb1_d[:], esb1[:])
                nc.sync.dma_start(dbg_esb2_d[:], esb2[:])

    nc.compile()
    return nc


def _prep_inputs(x, W1, attn1, W2, attn2, gamma, beta):
    f32 = np.float32
    x = np.asarray(x, f32)
    W1 = np.asarray(W1, f32)
    attn1 = np.asarray(attn1, f32)
    W2 = np.asarray(W2, f32)
    attn2 = np.asarray(attn2, f32)
    gamma = np.asarray(gamma, f32)
    beta = np.asarray(beta, f32)

    vsrc1 = np.stack([W1[:, h * D1:(h + 1) * D1] @ attn1[h, :D1]
                      for h in range(H)], 1)
    vtgt1 = np.stack([W1[:, h * D1:(h + 1) * D1] @ attn1[h, D1:]
                      for h in range(H)], 1)
    w1a = np.concatenate([W1, vsrc1, vtgt1], 1).astype(_BF)

    vsrc2 = np.stack([W2[:, h * HID:(h + 1) * HID] @ attn2[h, :HID]
                      for h in range(H)], 1)
    vtgt2 = np.stack([W2[:, h * HID:(h + 1) * HID] @ attn2[h, HID:]
                      for h in range(H)], 1)
    w2a = W2.astype(_BF)

    w2e_f = np.concatenate([vsrc2, vtgt2], 1)          # (256, 8)
    w2e8 = np.ascontiguousarray(
        (w2e_f * SW2E).reshape(2, K1, 2 * H).transpose(1, 0, 2)).astype(_F8)
    xT = np.ascontiguousarray(x.T).astype(_BF)
    gb = np.stack([gamma, beta], 0).astype(f32)

    in_maps = []
    for c in range(NCORES):
        xTm = np.ascontiguousarray(x[c * NS:(c + 1) * NS, :].T).astype(_BF)
        in_maps.append(dict(xT=xT, xTm=xTm, w1a=w1a, w2a=w2a,
                            w2e8=w2e8, gb=gb))
    return in_maps


def _ensure_ntff_hook():
    """Inject the antenv.axon_hooks shim (missing in this image) so
    run_bass_kernel_spmd(trace=True) can capture NTFF profiles via the
    axon .so's C ABI (same mechanism as trn_agent_boot)."""
    import sys
    import types
    import ctypes
    import contextlib

    if "antenv.axon_hooks" in sys.modules:
        return
    so_path = "/opt/axon/libaxon_pjrt.so"
    try:
        lib = ctypes.CDLL(so_path)
    except OSError:
        return
    if not hasattr(lib, "axon_start_nrt_profile"):
        return
    lib.axon_start_nrt_profile.argtypes = [ctypes.POINTER(ctypes.c_int64),
                                           ctypes.c_size_t]
    lib.axon_start_nrt_profile.restype = ctypes.c_int64
    lib.axon_stop_nrt_profile.argtypes = [ctypes.c_char_p]
    lib.axon_stop_nrt_profile.restype = ctypes.c_int64

    @contextlib.contextmanager
    def _hook(output_dir, device_ids):
        import jax
        jax.devices()
        if device_ids:
            ids = (ctypes.c_int64 * len(device_ids))(*device_ids)
            rc = lib.axon_start_nrt_profile(ids, len(device_ids))
        else:
            rc = lib.axon_start_nrt_profile(None, 0)
        if rc != 0:
            raise RuntimeError(f"axon_start_nrt_profile rc={rc}")
        try:
            yield
        finally:
            n = lib.axon_stop_nrt_profile(str(output_dir).encode())
            print(f"ntff profile: {n} file(s) written to {output_dir}")

    mod = types.ModuleType("antenv.axon_hooks")
    mod.get_axon_ntff_profile_hook = lambda: _hook
    mod.set_axon_ntff_profile_hook = lambda h: None
    sys.modules["antenv.axon_hooks"] = mod


def _run(in_maps, trace=False):
    global _compiled
    from concourse.bass_utils import run_bass_kernel_spmd
    if trace:
        _ensure_ntff_hook()
    if _compiled is None:
        _compiled = _build()
    res = run_bass_kernel_spmd(_compiled, in_maps,
                               core_ids=list(range(NCORES)), trace=trace)
    out = np.concatenate([res.results[c]["outT"] for c in range(NCORES)], 0)
    return out.astype(np.float32), res


def kernel(x, W1, attn1, W2, attn2, gamma, beta):
    in_maps = _prep_inputs(x, W1, attn1, W2, attn2, gamma, beta)
    out, _ = _run(in_maps, trace=False)
    return out


def kernel_traced(x, W1, attn1, W2, attn2, gamma, beta):
    """Like kernel() but returns (out, BassKernelResults) with profiling."""
    in_maps = _prep_inputs(x, W1, attn1, W2, attn2, gamma, beta)
    return _run(in_maps, trace=True)



# revision 53
# speedup vs baseline: 1.1655x; 1.1655x over previous
"""Trainium2 Bass kernel for a 2-layer dense-graph GAT encoder (N=4096, H=4).

Math: attention scores are additive: e[i,j,h] = lrelu(e_src[i,h] + e_tgt[j,h]).
exp(lrelu(s)) with s = es + et factors as
    exp(0.2*es) * [ c * max(1, u*v) ],   u = exp(0.8*es_i), v = exp(0.8*et_j),
    c = exp(0.2*et_j),
and the exp(0.2*es_i) factor cancels in the softmax.  So each (j,i) attention
tile is ONE DVE tensor_scalar op:  T[j,i] = max(c_j, (c_j*v_j)*u_i)  applied to
a broadcast tile of u — no N^2 transcendentals.  The N^2 work left is one DVE
op + one PE matmul per 128x512 tile.

Sharding: rows (queries) are split 512/core across 8 cores.  Layer-1
projections (x @ W1) are computed replicated from a pre-transposed x; the
layer-1 output shard h^T (256x512) is AllGathered between layers; layer-2
projections are recomputed replicated from the gathered h^T.  Final output is
returned per-core as (512, 256) row shards and concatenated on host.
"""

import numpy as np
import ml_dtypes

N = 4096
NCORES = 8
NS = N // NCORES          # 512 rows per core
H = 4
D1 = 64                   # layer-1 head dim
HID = 256                 # hidden = H*D1, layer-2 head dim
K1 = 128                  # state_dim
NT = N // 128             # 32 j-tiles
LN_EPS = 1e-5
SW2E = 16.0               # fp8 scale for the layer-2 score columns
WXS = 256.0               # fp8 scale for wx2 (layer-2 V-side)
S2 = 64.0                 # fp8 scale for T2 tiles (cancels in num/den)
# layer-2 j-tiles generated on ScalarE in relu/deviation form (13 of 32)
ACT2 = (1, 3, 6, 8, 11, 13, 16, 18, 21, 23, 26, 28, 31)

_BF = ml_dtypes.bfloat16
_F8 = ml_dtypes.float8_e4m3

_compiled = None
_DEBUG = False


def _build():
    import concourse.bass as bass
    import concourse.mybir as mybir
    import concourse.tile as tile
    from concourse import bacc

    # All ACT functions used here (Exp, Ln, Copy, Relu, Square, Identity)
    # live in the natural_log_exp_and_others set; prefer it so the table is
    # loaded once instead of thrashing Ln<->Exp sets in the LN tail.
    if not getattr(bacc, "_ant_act_tables_patched", False):
        _orig_gat = bacc.get_activation_tables

        def _pref_tables(arch):
            tabs = dict(_orig_gat(arch))
            pref = "natural_log_exp_and_others"
            if pref in tabs:
                # keep entry ORDER (act_func_set_id is positional) but hide
                # every other set's functions so the picker lands on pref
                tabs = {k: (v if k == pref else set())
                        for k, v in tabs.items()}
            return tabs

        bacc.get_activation_tables = _pref_tables
        bacc._ant_act_tables_patched = True

    f32 = mybir.dt.float32
    bf16 = mybir.dt.bfloat16
    f8 = mybir.dt.float8e4
    AF = mybir.ActivationFunctionType
    OP = mybir.AluOpType
    PM = mybir.MatmulPerfMode

    nc = bacc.Bacc("TRN2", target_bir_lowering=False, debug=False,
                   num_devices=NCORES)

    # ---- I/O ----
    xT_d = nc.dram_tensor("xT", [K1, N], bf16, kind="ExternalInput")
    xTm_d = nc.dram_tensor("xTm", [K1, NS], bf16, kind="ExternalInput")
    w1_d = nc.dram_tensor("w1a", [K1, HID + 2 * H], bf16, kind="ExternalInput")
    w2_d = nc.dram_tensor("w2a", [HID, H * HID], bf16, kind="ExternalInput")
    w2e8_d = nc.dram_tensor("w2e8", [K1, 2, 2 * H], f8, kind="ExternalInput")
    gb_d = nc.dram_tensor("gb", [2, HID], f32, kind="ExternalInput")
    out_d = nc.dram_tensor("outT", [NS, HID], f32, kind="ExternalOutput")
    if _DEBUG:
        dbg_esb1_d = nc.dram_tensor("dbg_esb1", [128, NT, 8], f32,
                                    kind="ExternalOutput")
        dbg_esb2_d = nc.dram_tensor("dbg_esb2", [128, NT, 8], f32,
                                    kind="ExternalOutput")
        dbg_h1_d = nc.dram_tensor("dbg_h1", [128, 2, NS], f32,
                                  kind="ExternalOutput")
        dbg_o2_d = nc.dram_tensor("dbg_o2", [NS, HID], f32,
                                  kind="ExternalOutput")

    W1C = HID + 2 * H        # 264
    W2C = H * HID + 2 * H    # 1032

    with tile.TileContext(nc) as tc:
        with (
            tc.tile_pool(name="persist", bufs=1) as pp,
            tc.tile_pool(name="xpool", bufs=1) as xp,
            tc.tile_pool(name="work", bufs=1) as wp,
            tc.tile_pool(name="tp", bufs=8) as tp,
            tc.tile_pool(name="dram", bufs=1, space="DRAM") as dram,
        ):
            # tiny dummy collective FIRST so the CC firmware warm-up
            # (barrier + HAM setup, ~55us) starts as early as possible and
            # runs under layer-1 compute.
            warm_in = dram.tile([1, 64], bf16)
            warm_out = dram.tile([NCORES, 64], bf16, addr_space="Shared")
            warm_sb = wp.tile([1, 64], bf16, tag="warm")
            nc.vector.memset(warm_sb[:], 0.0)
            nc.sync.dma_start(warm_in[:], warm_sb[:])
            nc.gpsimd.collective_compute(
                "AllGather", OP.bypass,
                replica_groups=[list(range(NCORES))],
                ins=[warm_in.opt()], outs=[warm_out.opt()])

            # ---------- loads ----------
            xT = xp.tile([K1, N], bf16)
            xTm = xp.tile([K1, NS], bf16)
            w1 = pp.tile([K1, W1C], bf16)
            w2 = pp.tile([K1, 2, H * HID], bf16)   # k-tiles of W2 rows
            nc.sync.dma_start(w1[:], w1_d[:])
            nc.sync.dma_start(xTm[:], xTm_d[:])
            for q in range(4):  # chunked for DMA parallelism
                nc.sync.dma_start(xT[:, q * 1024:(q + 1) * 1024],
                                  xT_d[:, q * 1024:(q + 1) * 1024])
            # w2/w2e8 are not needed until the layer-2 projection — load
            # them after the layer-1 operands so phase A starts sooner
            nc.sync.dma_start(w2[:, 0, :], w2_d[0:128, :])
            nc.sync.dma_start(w2[:, 1, :], w2_d[128:256, :])
            w2e8 = pp.tile([K1, 2, 2 * H], f8)
            nc.sync.dma_start(w2e8[:], w2e8_d[:])

            # gamma/beta broadcast rows
            g_row = pp.tile([1, HID], f32)
            b_row = pp.tile([1, HID], f32)
            nc.sync.dma_start(g_row[:], gb_d[0:1, :])
            nc.sync.dma_start(b_row[:], gb_d[1:2, :])
            g_brc = pp.tile([128, HID], f32)
            b_brc = pp.tile([128, HID], f32)
            nc.gpsimd.partition_broadcast(g_brc[:], g_row[:])
            nc.gpsimd.partition_broadcast(b_brc[:], b_row[:])

            # ---------- persistent layer-1 state ----------
            wx1 = pp.tile([128, NT, H, D1 + 1], bf16)     # [.., 0:64]=Wx, 64=ones
            nc.vector.memset(wx1[:, :, :, D1], 1.0)
            esb1 = wp.tile([128, NT, 8], f32, tag="esb1")
            c1 = pp.tile([128, NT, H], f32)
            cv1 = pp.tile([128, NT, H], f32)

            with tc.tile_pool(name="psA", bufs=4, space="PSUM") as psA:
                for jt in range(NT):
                    pA = psA.tile([128, W1C], f32, tag="pA")
                    nc.tensor.matmul(pA[:], xT[:, jt * 128:(jt + 1) * 128],
                                     w1[:], start=True, stop=True)
                    if jt % 2 == 0:
                        nc.vector.tensor_copy(wx1[:, jt, :, 0:D1],
                                              pA[:, 0:HID])
                    else:
                        nc.scalar.copy(wx1[:, jt, :, 0:D1], pA[:, 0:HID])
                    nc.vector.tensor_copy(esb1[:, jt, :], pA[:, HID:W1C])

                # u1 rows for my shard: e_src1^T via M=1 matmuls
                u1row = []
                for h in range(H):
                    pu = psA.tile([1, NS], f32, tag="pu", bufs=2)
                    nc.tensor.matmul(pu[:], w1[:, HID + h:HID + h + 1],
                                     xTm[:], start=True, stop=True)
                    ur = pp.tile([1, NS], bf16, name=f"u1row{h}",
                                 tag=f"u1row{h}")
                    nc.scalar.activation(ur[:], pu[:], AF.Exp, scale=0.8)
                    u1row.append(ur)

            uv1 = wp.tile([128, NT, 8], f32, tag="uv1")
            nc.scalar.activation(uv1[:], esb1[:], AF.Exp, scale=0.8)
            nc.scalar.activation(c1[:], esb1[:, :, H:2 * H], AF.Exp, scale=0.2)
            nc.vector.tensor_tensor(cv1[:], uv1[:, :, H:2 * H], c1[:], OP.mult)
            # for the ACT-generated (relu-form) head: -c bias + bf16 c column
            nc1 = pp.tile([128, NT, H], f32)
            nc.vector.tensor_scalar(nc1[:], c1[:], -1.0, None, OP.mult)
            c1b = pp.tile([128, NT, H], bf16)
            nc.vector.tensor_copy(c1b[:], c1[:])
            ones_row = pp.tile([1, NS], bf16)
            nc.vector.memset(ones_row[:], 1.0)

            bu1 = pp.tile([128, H, NS], bf16)
            for h in range(H):
                nc.gpsimd.partition_broadcast(bu1[:, h, :], u1row[h][:])

            # ---------- phase B: layer-1 attention for my 512 rows ----------
            # tiles with jt % 4 == 3 are generated on ScalarE in relu-form:
            #   R = relu(cv*u - c) = c*(max(1, u*v) - 1)
            # their matmuls miss sum_{those j} c_j*Wx_aug[j,:], injected via
            # a K=1 matmul of the per-head correction row C1_h.
            ACT_JT = [jt for jt in range(NT) if jt % 3 == 1]
            # single bounce for all 4 heads: [p, kt, i] (kt: head pair)
            bounce = dram.tile([128, 2, NS], f8)
            gat = dram.tile([NCORES, 128, 2, NS], f8, addr_space="Shared")
            hallT = pp.tile([128, 2, NCORES, NS], f8)
            with tc.tile_pool(name="psB", bufs=1, space="PSUM") as psB:
                # correction rows: C1_h = sum_{jt in ACT_JT} c_j*Wx_aug[j,:]
                c1s = []
                for h in range(H):
                    pC1 = psB.tile([1, D1 + 1], f32, tag="pC1", bufs=2)
                    for i, jt in enumerate(ACT_JT):
                        nc.tensor.matmul(pC1[:], c1b[:, jt, h:h + 1],
                                         wx1[:, jt, h, :],
                                         start=(i == 0),
                                         stop=(i == len(ACT_JT) - 1))
                    cs = wp.tile([1, D1 + 1], bf16, name=f"c1s{h}",
                                 tag=f"c1s{h}")
                    nc.vector.tensor_copy(cs[:], pC1[:])
                    c1s.append(cs)

                def _epi1(h, pB):
                    # epilogue: h1 = elu(num/den), DMA into bounce half
                    # o row 64 doubles as the den staging row
                    o = wp.tile([D1 + 1, NS], f32, tag="o", bufs=2)
                    nc.vector.tensor_copy(o[D1:D1 + 1, :], pB[D1:D1 + 1, :])
                    den = wp.tile([1, NS], f32, tag="den", bufs=2)
                    nc.sync.dma_start(den[:], o[D1:D1 + 1, :])
                    denr = wp.tile([1, NS], f32, tag="denr", bufs=2)
                    nc.vector.reciprocal_approx_fast(denr[:], den[:])
                    brc = wp.tile([D1, NS], f32, tag="brc", bufs=2)
                    nc.gpsimd.partition_broadcast(brc[:], denr[:])
                    nc.vector.tensor_tensor(o[0:D1, :], pB[0:D1, :], brc[:],
                                            OP.mult)
                    # elu(x) = (relu(x) - 1) + exp(min(x, 0))
                    tmn = wp.tile([D1, NS], f32, tag="tmn", bufs=2)
                    nc.vector.tensor_scalar(tmn[:], o[0:D1, :], 0.0, None,
                                            OP.min)
                    nc.scalar.activation(tmn[:], tmn[:], AF.Exp)  # in place
                    trl = wp.tile([D1, NS], f32, tag="trl", bufs=2)
                    nc.vector.tensor_scalar(trl[:], o[0:D1, :], 0.0, -1.0,
                                            OP.max, OP.add)
                    eluh = wp.tile([D1, NS], f8, name=f"eluh{h}",
                                   tag=f"eluh{h}")
                    nc.vector.tensor_tensor(eluh[:], tmn[:], trl[:], OP.add)
                    nc.sync.dma_start(
                        bounce[(h % 2) * D1:(h % 2 + 1) * D1, h // 2, :],
                        eluh[:])
                    # one gather for all heads: avoids the ~8us CC
                    # firmware gap between back-to-back collectives
                    if h == 3:
                        nc.gpsimd.collective_compute(
                            "AllGather", OP.bypass,
                            replica_groups=[list(range(NCORES))],
                            ins=[bounce.opt()], outs=[gat.opt()])
                        # per-source-core reshuffle so the projection's
                        # first j-tiles (core 0) start before the whole
                        # 1MB lands
                        for cc in range(NCORES):
                            nc.sync.dma_start(hallT[:, :, cc, :], gat[cc])

                for h in range(H):
                    pB = psB.tile([D1 + 1, NS], f32, name=f"pB{h}",
                                  tag=f"pB{h}")
                    # inject correction row (opens the accumulation group)
                    nc.tensor.matmul(pB[:], c1s[h][:], ones_row[:],
                                     start=True, stop=False)
                    for jt in range(NT):
                        t1 = tp.tile([128, NS], bf16, tag="T1")
                        if jt % 3 == 1:
                            nc.scalar.activation(
                                t1[:], bu1[:, h, :], AF.Relu,
                                bias=nc1[:, jt, h:h + 1],
                                scale=cv1[:, jt, h:h + 1])
                        else:
                            nc.vector.tensor_scalar(
                                t1[:], bu1[:, h, :], cv1[:, jt, h:h + 1],
                                c1[:, jt, h:h + 1], OP.mult, OP.max)
                        nc.tensor.matmul(
                            pB[:], wx1[:, jt, h, :], t1[:],
                            start=False,
                            stop=(jt == NT - 1))
                    _epi1(h, pB)

            # my own h^T back from local bounce (for u2 rows)
            hmT = wp.tile([128, 2, NS], f8)
            nc.sync.dma_start(hmT[:], bounce[:])

            if _DEBUG:
                dbg_h1 = wp.tile([128, 2, NS], f8)
                nc.sync.dma_start(dbg_h1[:], bounce[:])
                dbg_h1f = wp.tile([128, 2, NS], f32)
                nc.vector.tensor_copy(dbg_h1f[:], dbg_h1[:])
                nc.sync.dma_start(dbg_h1_d[:], dbg_h1f[:])

            # ---------- persistent layer-2 state ----------
            # wx2 holds Wx2_aug * WXS in fp8, laid out for DoubleRow matmuls:
            # [p, jt2, k, h, col] = Wx2_aug[jt2*256 + k*128 + p, h, col]
            NT2 = NT // 2
            wx2 = pp.tile([128, NT2, 2, H, HID + 1], f8)
            nc.vector.memset(wx2[:, :, :, :, HID], 1.0)  # den col: NOT scaled
            esb2 = wp.tile([128, NT, 8], f32, tag="esb1")
            c2 = pp.tile([128, NT, H], f32)
            cv2 = pp.tile([128, NT, H], f32)

            # u2 rows from local h^T (own pool scope to free the bank
            # before the proj loop fills PSUM)
            u2row = []
            with tc.tile_pool(name="psU", bufs=1, space="PSUM") as psU:
                for h in range(H):
                    pu2 = psU.tile([1, NS], f32, tag="pu2", bufs=1)
                    for kt in range(2):
                        nc.tensor.matmul(pu2[:], w2e8[:, kt, h:h + 1],
                                         hmT[:, kt, :], start=(kt == 0),
                                         stop=(kt == 1))
                    ur2 = pp.tile([1, NS], bf16, name=f"u2row{h}",
                                  tag=f"u2row{h}")
                    nc.scalar.activation(ur2[:], pu2[:], AF.Exp,
                                         scale=0.8 / SW2E)
                    u2row.append(ur2)

            bu2 = pp.tile([128, H, NS], bf16)
            for h in range(H):
                nc.gpsimd.partition_broadcast(bu2[:, h, :], u2row[h][:])

            # keep-warm: the PE idles ~25us during the AllGather, which
            # drops the HAM clock gate to K=4/8 and makes the first ~8us of
            # the projection run at half clock.  Burn junk matmuls (gated on
            # hmT so they can't start before phase B ends) to hold K=8/8.
            with tc.tile_pool(name="psW", bufs=1, space="PSUM") as psW:
                pw = psW.tile([128, NS], f32, tag="pw", bufs=2)
                for i in range(88):
                    nc.tensor.matmul(pw[:], hmT[:, 0, 0:128], hmT[:, 0, :],
                                     start=True, stop=True)

            # per-jt attention scalars, filled chunk-wise during the proj
            # loop so phase-E T-gen can start while proj is still running
            uv2 = wp.tile([128, NT, 8], f32, tag="uv1")
            cvS2 = pp.tile([128, NT, H], f32)
            cS2 = pp.tile([128, NT, H], f32)
            ncS2 = pp.tile([128, NT, H], f32)
            cS2b = pp.tile([128, NT, H], bf16)
            cs4 = wp.tile([1, H, HID + 1], bf16)  # ACT-set correction rows

            with tc.tile_pool(name="psD", bufs=1, space="PSUM") as psD:
                # Wx2_aug replicated: all 4096 rows
                for jt in range(NT):
                    c8, io = divmod(jt, NT // NCORES)
                    jt2, kk = divmod(jt, 2)
                    pD1 = psD.tile([128, 2, HID], f32, tag="pD1", bufs=3)
                    pD2 = psD.tile([128, 2, HID], f32, tag="pD2", bufs=3)
                    pD3 = psD.tile([128, 8], f32, tag="pD3", bufs=2)
                    lhs2 = hallT[:, :, c8, io * 128:(io + 1) * 128]
                    for kt in range(2):
                        lhs = hallT[:, kt, c8, io * 128:(io + 1) * 128]
                        st, sp = (kt == 0), (kt == 1)
                        nc.tensor.matmul(pD1[:], lhs, w2[:, kt, 0:512],
                                         start=st, stop=sp)
                        nc.tensor.matmul(pD2[:], lhs, w2[:, kt, 512:1024],
                                         start=st, stop=sp)
                    nc.tensor.matmul(pD3[:], lhs2, w2e8[:],
                                     start=True, stop=True,
                                     perf_mode=PM.DoubleRow)
                    # esb2 copy first so pD3 recycles promptly
                    nc.vector.tensor_copy(esb2[:, jt, :], pD3[:])
                    if jt % 2 == 0:
                        nc.vector.tensor_scalar(wx2[:, jt2, kk, 0:2, 0:HID],
                                                pD1[:], WXS, None, OP.mult)
                        nc.scalar.mul(wx2[:, jt2, kk, 2:4, 0:HID], pD2[:],
                                      WXS)
                    else:
                        nc.scalar.mul(wx2[:, jt2, kk, 0:2, 0:HID], pD1[:],
                                      WXS)
                        nc.vector.tensor_scalar(wx2[:, jt2, kk, 2:4, 0:HID],
                                                pD2[:], WXS, None, OP.mult)
                    if jt % 8 == 7:
                        # chunk of 8 jt finished: derive attention scalars
                        # (esb2 carries an extra SW2E factor from w2e8)
                        sl = slice(jt - 7, jt + 1)
                        nc.scalar.activation(uv2[:, sl, :], esb2[:, sl, :],
                                             AF.Exp, scale=0.8 / SW2E)
                        nc.scalar.activation(c2[:, sl, :],
                                             esb2[:, sl, H:2 * H],
                                             AF.Exp, scale=0.2 / SW2E)
                        nc.vector.tensor_tensor(cv2[:, sl, :],
                                                uv2[:, sl, H:2 * H],
                                                c2[:, sl, :], OP.mult)
                        nc.vector.tensor_scalar(cvS2[:, sl, :], cv2[:, sl, :],
                                                S2, None, OP.mult)
                        nc.vector.tensor_scalar(cS2[:, sl, :], c2[:, sl, :],
                                                S2, None, OP.mult)
                        nc.vector.tensor_scalar(ncS2[:, sl, :], c2[:, sl, :],
                                                -S2, None, OP.mult)
                        nc.vector.tensor_copy(cS2b[:, sl, :], cS2[:, sl, :])

            # correction rows: C2_h = sum_{jt in ACT2} S2*c_j * wx2[j,h,:]
            # (consumed by the inject-last matmuls, so running these after
            # the proj loop costs nothing on the critical path)
            with tc.tile_pool(name="psC", bufs=1, space="PSUM") as psC:
                for h in range(H):
                    pC2 = psC.tile([1, HID + 1], f32, tag="pC2", bufs=2)
                    for i, ja in enumerate(ACT2):
                        nc.tensor.matmul(pC2[:], cS2b[:, ja, h:h + 1],
                                         wx2[:, ja // 2, ja % 2, h, :],
                                         start=(i == 0),
                                         stop=(i == len(ACT2) - 1))
                    nc.vector.tensor_copy(cs4[:, h, :], pC2[:])

            # ---------- phase E: layer-2 attention + LN ----------
            # head-outer, full i width; per head 4 psum tiles (one per
            # 128-row i-subtile); two heads' tile sets rotate in PSUM.
            # fp8 T2 (scaled by S2) against fp8 wx2 (scaled by WXS) with
            # DoubleRow j-pair matmuls; jt in ACT2 generated on ScalarE in
            # relu (deviation) form, rest on DVE in direct max form.
            o2acc = [wp.tile([128, HID], f32, name=f"o2acc{m}",
                             tag=f"o2acc{m}") for m in range(4)]
            eps_c = pp.tile([128, 1], f32)
            nc.vector.memset(eps_c[:], LN_EPS)

            def _tail(m):
                # elu + layernorm over features for one 128-row m-chunk.
                # exp(min(x,0)) = exp(-relu(-x)): two ACT ops, no DVE min;
                # mean/var via bn_stats/bn_aggr in two DVE ops.
                o2 = o2acc[m]
                eex = wp.tile([128, HID], f32, tag="eex", bufs=2)
                nc.scalar.activation(eex[:], o2[:], AF.Relu, scale=-1.0)
                nc.scalar.activation(eex[:], eex[:], AF.Exp, scale=-1.0)
                erl = wp.tile([128, HID], f32, tag="erl", bufs=2)
                nc.vector.tensor_scalar(erl[:], o2[:], 0.0, -1.0,
                                        OP.max, OP.add)
                he = wp.tile([128, HID], f32, tag="he", bufs=2)
                nc.vector.tensor_tensor(he[:], eex[:], erl[:], OP.add)
                st6 = wp.tile([128, 6], f32, tag="st6", bufs=2)
                nc.vector.bn_stats(st6[:], he[:])
                mv = wp.tile([128, 2], f32, tag="mv", bufs=2)
                nc.vector.bn_aggr(mv[:], st6[:])
                lnv = wp.tile([128, 1], f32, tag="lnv", bufs=2)
                nc.scalar.activation(lnv[:], mv[:, 1:2], AF.Ln,
                                     bias=eps_c[:])
                rstd = wp.tile([128, 1], f32, tag="rstd", bufs=2)
                nc.scalar.activation(rstd[:], lnv[:], AF.Exp, scale=-0.5)
                xn = wp.tile([128, HID], f32, tag="xn", bufs=2)
                nc.vector.tensor_scalar(xn[:], he[:], mv[:, 0:1], rstd[:],
                                        OP.subtract, OP.mult)
                y = wp.tile([128, HID], f32, tag="y", bufs=2)
                nc.vector.tensor_tensor(y[:], xn[:], g_brc[:], OP.mult)
                outt = wp.tile([128, HID], f32, tag="outt", bufs=2)
                nc.vector.tensor_tensor(outt[:], y[:], b_brc[:], OP.add)
                r0 = m * 128
                nc.sync.dma_start(out_d[r0:r0 + 128, :], outt[:])
                if _DEBUG:
                    nc.sync.dma_start(dbg_o2_d[r0:r0 + 128, :], o2[:])

            with tc.tile_pool(name="psE", bufs=1, space="PSUM") as psE:
                def _gen_row(h):
                    # T2 tiles for one head (issued one head ahead so the
                    # last head's LN chains never queue behind T-gen)
                    row = []
                    for jt2 in range(NT2):
                        t2 = tp.tile([128, 2, NS], f8, tag="T2", bufs=32)
                        row.append(t2)
                        for k in range(2):
                            jt = jt2 * 2 + k
                            if jt in ACT2:
                                nc.scalar.activation(
                                    t2[:, k, :], bu2[:, h, :], AF.Relu,
                                    bias=ncS2[:, jt, h:h + 1],
                                    scale=cvS2[:, jt, h:h + 1])
                            else:
                                nc.vector.tensor_scalar(
                                    t2[:, k, :], bu2[:, h, :],
                                    cvS2[:, jt, h:h + 1],
                                    cS2[:, jt, h:h + 1], OP.mult, OP.max)
                    return row

                nextrow = _gen_row(0)
                for h in range(H):
                    pE = [psE.tile([128, HID + 1], f32,
                                   name=f"pE{h}_{m}", tag=f"pE{h % 2}_{m}")
                          for m in range(4)]
                    t2s = nextrow
                    if h < H - 1:
                        for jt2 in range(NT2):
                            for m in range(4):
                                nc.tensor.matmul(
                                    pE[m][:],
                                    t2s[jt2][:, :, m * 128:(m + 1) * 128],
                                    wx2[:, jt2, :, h, :],
                                    start=(jt2 == 0), stop=False,
                                    perf_mode=PM.DoubleRow)
                        nextrow = _gen_row(h + 1)
                    if h == H - 1:
                        # last head goes m-major so each m-chunk's inject,
                        # epilogue and LN tail overlap the next chunk's
                        # matmuls instead of all landing after the last one
                        for m in range(4):
                            for jt2 in range(NT2):
                                nc.tensor.matmul(
                                    pE[m][:],
                                    t2s[jt2][:, :, m * 128:(m + 1) * 128],
                                    wx2[:, jt2, :, h, :],
                                    start=(jt2 == 0), stop=False,
                                    perf_mode=PM.DoubleRow)
                            nc.tensor.matmul(pE[m][:], ones_row[:, 0:128],
                                             cs4[:, h, :], start=False,
                                             stop=True)
                            dpk1 = wp.tile([128, 1], f32, tag="dpk1",
                                           bufs=2)
                            nc.vector.tensor_scalar(
                                dpk1[:], pE[m][:, HID:HID + 1],
                                float(H) * WXS, None, OP.mult)
                            rr1 = wp.tile([128, 1], f32, tag="rr1", bufs=2)
                            nc.vector.reciprocal_approx_fast(rr1[:],
                                                             dpk1[:])
                            nc.vector.scalar_tensor_tensor(
                                o2acc[m][:], pE[m][:, 0:HID],
                                rr1[:], o2acc[m][:], OP.mult, OP.add)
                            _tail(m)
                        continue
                    # inject the ACT-set correction row last (closes the
                    # group) — lets the jt2 matmuls start before the
                    # correction rows are ready
                    for m in range(4):
                        nc.tensor.matmul(pE[m][:], ones_row[:, 0:128],
                                         cs4[:, h, :], start=False,
                                         stop=True)
                    # divide by H*WXS*den and accumulate the head mean
                    dpk = wp.tile([128, 4], f32, tag="dpk", bufs=2)
                    for m in range(4):
                        nc.vector.tensor_scalar(
                            dpk[:, m:m + 1], pE[m][:, HID:HID + 1],
                            float(H) * WXS, None, OP.mult)
                    rr = wp.tile([128, 4], f32, tag="rr", bufs=2)
                    nc.vector.reciprocal_approx_fast(rr[:], dpk[:])
                    for m in range(4):
                        if h == 0:
                            nc.vector.tensor_scalar(
                                o2acc[m][:], pE[m][:, 0:HID],
                                rr[:, m:m + 1], None, OP.mult)
                        else:
                            nc.vector.scalar_tensor_tensor(
                                o2acc[m][:], pE[m][:, 0:HID],
                                rr[:, m:m + 1], o2acc[m][:],
                                OP.mult, OP.add)
            if _DEBUG:
                nc.sync.dma_start(dbg_esb1_d[:], esb1[:])
                nc.sync.dma_start(dbg_esb2_d[:], esb2[:])

    nc.compile()
    return nc


def _prep_inputs(x, W1, attn1, W2, attn2, gamma, beta):
    f32 = np.float32
    x = np.asarray(x, f32)
    W1 = np.asarray(W1, f32)
    attn1 = np.asarray(attn1, f32)
    W2 = np.asarray(W2, f32)
    attn2 = np.asarray(attn2, f32)
    gamma = np.asarray(gamma, f32)
    beta = np.asarray(beta, f32)

    vsrc1 = np.stack([W1[:, h * D1:(h + 1) * D1] @ attn1[h, :D1]
                      for h in range(H)], 1)
    vtgt1 = np.stack([W1[:, h * D1:(h + 1) * D1] @ attn1[h, D1:]
                      for h in range(H)], 1)
    w1a = np.concatenate([W1, vsrc1, vtgt1], 1).astype(_BF)

    vsrc2 = np.stack([W2[:, h * HID:(h + 1) * HID] @ attn2[h, :HID]
                      for h in range(H)], 1)
    vtgt2 = np.stack([W2[:, h * HID:(h + 1) * HID] @ attn2[h, HID:]
                      for h in range(H)], 1)
    w2a = W2.astype(_BF)

    w2e_f = np.concatenate([vsrc2, vtgt2], 1)          # (256, 8)
    w2e8 = np.ascontiguousarray(
        (w2e_f * SW2E).reshape(2, K1, 2 * H).transpose(1, 0, 2)).astype(_F8)
    xT = np.ascontiguousarray(x.T).astype(_BF)
    gb = np.stack([gamma, beta], 0).astype(f32)

    in_maps = []
    for c in range(NCORES):
        xTm = np.ascontiguousarray(x[c * NS:(c + 1) * NS, :].T).astype(_BF)
        in_maps.append(dict(xT=xT, xTm=xTm, w1a=w1a, w2a=w2a,
                            w2e8=w2e8, gb=gb))
    return in_maps


def _ensure_ntff_hook():
    """Inject the antenv.axon_hooks shim (missing in this image) so
    run_bass_kernel_spmd(trace=True) can capture NTFF profiles via the
    axon .so's C ABI (same mechanism as trn_agent_boot)."""
    import sys
    import types
    import ctypes
    import contextlib

    if "antenv.axon_hooks" in sys.modules:
        return
    so_path = "/opt/axon/libaxon_pjrt.so"
    try:
        lib = ctypes.CDLL(so_path)
    except OSError:
        return
    if not hasattr(lib, "axon_start_nrt_profile"):
        return
    lib.axon_start_nrt_profile.argtypes = [ctypes.POINTER(ctypes.c_int64),
                                           ctypes.c_size_t]
    lib.axon_start_nrt_profile.restype = ctypes.c_int64
    lib.axon_stop_nrt_profile.argtypes = [ctypes.c_char_p]
    lib.axon_stop_nrt_profile.restype = ctypes.c_int64

    @contextlib.contextmanager
    def _hook(output_dir, device_ids):
        import jax
        jax.devices()
        if device_ids:
            ids = (ctypes.c_int64 * len(device_ids))(*device_ids)
            rc = lib.axon_start_nrt_profile(ids, len(device_ids))
        else:
            rc = lib.axon_start_nrt_profile(None, 0)
        if rc != 0:
            raise RuntimeError(f"axon_start_nrt_profile rc={rc}")
        try:
            yield
        finally:
            n = lib.axon_stop_nrt_profile(str(output_dir).encode())
            print(f"ntff profile: {n} file(s) written to {output_dir}")

    mod = types.ModuleType("antenv.axon_hooks")
    mod.get_axon_ntff_profile_hook = lambda: _hook
    mod.set_axon_ntff_profile_hook = lambda h: None
    sys.modules["antenv.axon_hooks"] = mod


def _run(in_maps, trace=False):
    global _compiled
    from concourse.bass_utils import run_bass_kernel_spmd
    if trace:
        _ensure_ntff_hook()
    if _compiled is None:
        _compiled = _build()
    res = run_bass_kernel_spmd(_compiled, in_maps,
                               core_ids=list(range(NCORES)), trace=trace)
    out = np.concatenate([res.results[c]["outT"] for c in range(NCORES)], 0)
    return out.astype(np.float32), res


def kernel(x, W1, attn1, W2, attn2, gamma, beta):
    in_maps = _prep_inputs(x, W1, attn1, W2, attn2, gamma, beta)
    out, _ = _run(in_maps, trace=False)
    return out


def kernel_traced(x, W1, attn1, W2, attn2, gamma, beta):
    """Like kernel() but returns (out, BassKernelResults) with profiling."""
    in_maps = _prep_inputs(x, W1, attn1, W2, attn2, gamma, beta)
    return _run(in_maps, trace=True)

